# revision 1
# baseline (speedup 1.0000x reference)
"""GCN layer kernel for Trainium2 (8 NeuronCores).

Strategy (per sharding_hint): shard node rows across the 8 cores for the
dense projection Z = X @ W (the FLOP-heavy part) on the TensorEngines.
W [512,512] is replicated. To avoid on-device transposes, the host passes
X^T and the device computes OutT = W^T @ X^T; the host transposes back.
The irregular COO scatter-add (segment_sum over 800k random edges) is done
host-side as a CSR SpMM, followed by ReLU.
"""

import numpy as np

N_NODES = 50000
M_IN = 512
H_OUT = 512
N_CORES = 8
# per-core padded column count: 13 blocks of 512
COLS = 6656
PAD_NODES = COLS * N_CORES  # 53248

_compiled = {}


def _build_nc():
    from concourse import bacc, mybir
    from concourse import tile

    f32 = mybir.dt.float32
    bf16 = mybir.dt.bfloat16
    nc = bacc.Bacc(None, debug=False)

    xt = nc.declare_dram_parameter("xt", [M_IN, COLS], bf16, isOutput=False)
    w = nc.declare_dram_parameter("w", [M_IN, H_OUT], bf16, isOutput=False)
    outt = nc.declare_dram_parameter("out", [H_OUT, COLS], f32, isOutput=True)

    KC = M_IN // 128  # 4 contraction chunks
    NC_ = H_OUT // 128  # 4 output-row chunks
    NB = COLS // 512  # 13 column blocks

    with tile.TileContext(nc) as tc:
        with (
            tc.tile_pool(name="wpool", bufs=1) as wpool,
            tc.tile_pool(name="xpool", bufs=3) as xpool,
            tc.tile_pool(name="opool", bufs=4) as opool,
            tc.tile_pool(name="psum", bufs=4, space="PSUM") as pp,
        ):
            wt = wpool.tile([128, KC, H_OUT], bf16)
            for k in range(KC):
                nc.sync.dma_start(wt[:, k, :], w[k * 128 : (k + 1) * 128, :])

            for cb in range(NB):
                xtile = xpool.tile([128, KC, 512], bf16)
                for k in range(KC):
                    nc.sync.dma_start(
                        xtile[:, k, :],
                        xt[k * 128 : (k + 1) * 128, cb * 512 : (cb + 1) * 512],
                    )
                for n in range(NC_):
                    acc = pp.tile([128, 512], f32)
                    for k in range(KC):
                        nc.tensor.matmul(
                            acc[:],
                            wt[:, k, n * 128 : (n + 1) * 128],
                            xtile[:, k, :],
                            start=(k == 0),
                            stop=(k == KC - 1),
                        )
                    otile = opool.tile([128, 512], f32)
                    nc.vector.tensor_copy(otile[:], acc[:])
                    nc.sync.dma_start(
                        outt[n * 128 : (n + 1) * 128, cb * 512 : (cb + 1) * 512],
                        otile[:],
                    )
    nc.compile()
    return nc


def _get_nc():
    if "nc" not in _compiled:
        _compiled["nc"] = _build_nc()
    return _compiled["nc"]


def kernel(X, W, edge_src, edge_dst, edge_vals):
    import scipy.sparse as sp
    from concourse.bass_utils import run_bass_kernel_spmd

    X = np.asarray(X, dtype=np.float32)
    W = np.ascontiguousarray(np.asarray(W, dtype=np.float32))
    edge_src = np.asarray(edge_src)
    edge_dst = np.asarray(edge_dst)
    edge_vals = np.asarray(edge_vals, dtype=np.float32)

    import ml_dtypes

    # host pre-transpose + pad so the device needs no transposes; bf16 halves
    # the upload and uses the fast TensorEngine path (rel err ~2e-3)
    bf = ml_dtypes.bfloat16
    XT = np.zeros((M_IN, PAD_NODES), dtype=bf)
    XT[:, :N_NODES] = X.T.astype(bf)
    W = W.astype(bf)

    in_maps = [
        {"xt": np.ascontiguousarray(XT[:, i * COLS : (i + 1) * COLS]), "w": W}
        for i in range(N_CORES)
    ]

    nc = _get_nc()
    res = run_bass_kernel_spmd(nc, in_maps, core_ids=list(range(N_CORES)))
    outs = res.results
    ZT = np.concatenate([np.asarray(outs[i]["out"]) for i in range(N_CORES)], axis=1)
    Z = np.ascontiguousarray(ZT[:, :N_NODES].T)  # [N, H]

    A = sp.csr_matrix(
        (edge_vals, (edge_dst.astype(np.int64), edge_src.astype(np.int64))),
        shape=(N_NODES, N_NODES),
    )
    agg = A @ Z
    return np.maximum(agg, 0.0).astype(np.float32)



# revision 8
# speedup vs baseline: 18.0926x; 18.0926x over previous
"""GCN layer kernel for Trainium2 (8 NeuronCores) — full computation on device.

Z = X @ W; agg = segment_sum(Z[edge_src] * edge_vals, edge_dst); out = relu(agg).

Strategy (per sharding_hint): shard DESTINATION nodes across the 8 cores.
Each core:
  phase 1 — computes the full dense projection Z = X @ W (replicated; the
    26 GFLOP matmul is cheap next to link traffic, so replication beats an
    all-gather halo exchange) and stores Z to local DRAM in bf16.
  phase 2 — processes the ~100k edges whose dst it owns, pre-sorted by dst
    tile on the host: for each 128-row dst tile, Q7 dma_gather fetches the
    Z rows of its source nodes (two calls, since int16 gather indices limit
    a table to 32k rows -> Z is split at row 25088), then a per-block
    selection matrix S (built on-device as (iota == slot) * edge_weight)
    folds the segment-sum into PSUM-accumulated matmuls: agg_tile += S^T @ G.
    ReLU on the way out.
Host does only graph partitioning/layout prep (sort edges by dst tile, pad
to 128-edge blocks) and the final unpad/concat.
"""

import numpy as np

N_NODES = 50000
M_IN = 512
H_OUT = 512
N_CORES = 8
P = 128
DPC = N_NODES // N_CORES          # 6250 dst nodes per core
NT = (DPC + P - 1) // P           # 49 dst tiles per core
PAD_DST = NT * P                  # 6272
NCH = 98                          # projection chunks of 512 nodes
NPAD = NCH * 512                  # 50176 padded nodes
KC = M_IN // P                    # 4 contraction chunks
SPLIT = NPAD // 2                 # 25088: Z half size (int16 gather indices)

_compiled = {}


def _build_nc(NBL, NBH):
    from contextlib import ExitStack
    from concourse import bacc, mybir
    from concourse import tile

    f32 = mybir.dt.float32
    bf16 = mybir.dt.bfloat16
    i16 = mybir.dt.int16
    NBT = NBL + NBH

    nc = bacc.Bacc(None, debug=False)

    xt = nc.declare_dram_parameter("xt", [M_IN, NPAD], bf16, isOutput=False)
    w = nc.declare_dram_parameter("w", [M_IN, H_OUT], bf16, isOutput=False)
    eidxlo = nc.declare_dram_parameter(
        "eidxlo", [NT * P, NBL * 8], i16, isOutput=False
    )
    eidxhi = nc.declare_dram_parameter(
        "eidxhi", [NT * P, NBH * 8], i16, isOutput=False
    )
    eslot = nc.declare_dram_parameter("eslot", [NT * P, NBT], f32, isOutput=False)
    ew = nc.declare_dram_parameter("ew", [NT * P, NBT], f32, isOutput=False)
    out = nc.declare_dram_parameter("out", [PAD_DST, H_OUT], f32, isOutput=True)

    # Z in two tensors: int16 gather indices address <32k rows each, and the
    # lo-half gathers can start while the hi half is still being projected.
    zlo = nc.dram_tensor("zlo", [SPLIT, H_OUT], bf16, kind="Internal")
    zhi = nc.dram_tensor("zhi", [SPLIT, H_OUT], bf16, kind="Internal")

    with tile.TileContext(nc) as tc:
        with ExitStack() as ctx:
            wpool = ctx.enter_context(tc.tile_pool(name="wpool", bufs=1))
            xpool = ctx.enter_context(tc.tile_pool(name="xpool", bufs=3))
            zpool = ctx.enter_context(tc.tile_pool(name="zpool", bufs=4))
            gpool = ctx.enter_context(tc.tile_pool(name="gpool", bufs=3))
            spool = ctx.enter_context(tc.tile_pool(name="spool", bufs=4))
            apool = ctx.enter_context(tc.tile_pool(name="apool", bufs=3))
            opool = ctx.enter_context(tc.tile_pool(name="opool", bufs=4))
            zpp = ctx.enter_context(tc.tile_pool(name="zpp", bufs=4, space="PSUM"))
            app = ctx.enter_context(tc.tile_pool(name="app", bufs=4, space="PSUM"))

            # ---- constants ----
            wsb = wpool.tile([P, KC, H_OUT], bf16)
            for k in range(KC):
                nc.sync.dma_start(wsb[:, k, :], w[k * P : (k + 1) * P, :])
            iota = wpool.tile([P, P], f32, tag="iota")
            nc.gpsimd.iota(
                iota[:], pattern=[[1, P]], base=0, channel_multiplier=0,
                allow_small_or_imprecise_dtypes=True,
            )

            # ---- phase 1: Z = X @ W, stored [NPAD, H] bf16 ----
            for c in range(NCH):
                n0 = c * 512
                xtile = xpool.tile([P, KC, 512], bf16)
                for k in range(KC):
                    nc.sync.dma_start(
                        xtile[:, k, :], xt[k * P : (k + 1) * P, n0 : n0 + 512]
                    )
                for s in range(4):
                    acc = zpp.tile([P, H_OUT], f32)
                    for k in range(KC):
                        nc.tensor.matmul(
                            acc[:],
                            xtile[:, k, s * P : (s + 1) * P],
                            wsb[:, k, :],
                            start=(k == 0),
                            stop=(k == KC - 1),
                        )
                    ztile = zpool.tile([P, H_OUT], bf16)
                    nc.vector.tensor_copy(ztile[:], acc[:])
                    zdst = zlo if n0 < SPLIT else zhi
                    zr0 = n0 - (0 if n0 < SPLIT else SPLIT) + s * P
                    nc.sync.dma_start(zdst[zr0 : zr0 + P, :], ztile[:])

            # ---- phase 2: per dst tile, gather + selection matmuls ----
            for t in range(NT):
                r0 = t * P
                idxlo_sb = apool.tile([P, NBL * 8], i16, tag="idxlo")
                nc.sync.dma_start(idxlo_sb[:], eidxlo[r0 : r0 + P, :])
                idxhi_sb = apool.tile([P, NBH * 8], i16, tag="idxhi")
                nc.sync.dma_start(idxhi_sb[:], eidxhi[r0 : r0 + P, :])
                slot_sb = apool.tile([P, NBT], f32, tag="slot")
                nc.sync.dma_start(slot_sb[:], eslot[r0 : r0 + P, :])
                w_sb = apool.tile([P, NBT], f32, tag="ew")
                nc.sync.dma_start(w_sb[:], ew[r0 : r0 + P, :])

                g = gpool.tile([P, NBT, H_OUT], bf16)
                # dma_gather breaks on HW above ~512 indices per call (ring
                # capacity); chunk into <=4-block (512-idx) calls.
                CH = 4
                for half, nb, zsrc, isb, off in (
                    (0, NBL, zlo, idxlo_sb, 0),
                    (1, NBH, zhi, idxhi_sb, NBL),
                ):
                    for c0 in range(0, nb, CH):
                        cn = min(CH, nb - c0)
                        nc.gpsimd.dma_gather(
                            g[:, off + c0 : off + c0 + cn, :],
                            zsrc[:],
                            isb[:, c0 * 8 : (c0 + cn) * 8],
                            cn * P,
                            cn * P,
                            H_OUT,
                        )

                acc = app.tile([P, H_OUT], f32)
                for b in range(NBT):
                    s = spool.tile([P, P], bf16)
                    nc.vector.tensor_scalar(
                        s[:], iota[:], slot_sb[:, b : b + 1], w_sb[:, b : b + 1],
                        mybir.AluOpType.is_equal, mybir.AluOpType.mult,
                    )
                    nc.tensor.matmul(
                        acc[:], s[:], g[:, b, :], start=(b == 0), stop=(b == NBT - 1)
                    )
                o = opool.tile([P, H_OUT], f32)
                nc.vector.tensor_scalar_max(o[:], acc[:], 0.0)
                nc.sync.dma_start(out[r0 : r0 + P, :], o[:])

    nc.compile()
    return nc


def _get_nc(NBL, NBH):
    if (NBL, NBH) not in _compiled:
        _compiled[(NBL, NBH)] = _build_nc(NBL, NBH)
    return _compiled[(NBL, NBH)]


def _wrap_idx16(vals, n_groups, nb):
    """[n_groups, nb*128] linear gather indices -> [n_groups*128, nb*8] int16
    in the Q7 wrapped layout (idx i at [i%16, i//16], replicated to all 8
    groups of 16 partitions)."""
    wr = vals.reshape(n_groups, nb * 8, 16).transpose(0, 2, 1)  # [G, 16, nb*8]
    rep = np.tile(wr, (1, 8, 1))  # [G, 128, nb*8]
    return np.ascontiguousarray(rep.reshape(n_groups * P, nb * 8))


def prepare(X, W, edge_src, edge_dst, edge_vals):
    """Host-side layout prep. Returns (nc, in_maps)."""
    import ml_dtypes

    bf = ml_dtypes.bfloat16
    X = np.asarray(X, dtype=np.float32)
    W = np.ascontiguousarray(np.asarray(W, dtype=np.float32))
    src = np.asarray(edge_src).astype(np.int64)
    dst = np.asarray(edge_dst).astype(np.int64)
    ev = np.asarray(edge_vals, dtype=np.float32)
    E = src.shape[0]

    XT = np.zeros((M_IN, NPAD), dtype=bf)
    XT[:, :N_NODES] = X.T.astype(bf)
    Wb = W.astype(bf)

    owner = dst // DPC
    localdst = dst - owner * DPC
    gtile = owner * NT + localdst // P       # 0..391 global dst tile
    slot = (localdst % P).astype(np.float32)
    half = (src >= SPLIT).astype(np.int64)   # 0 = lo table, 1 = hi table
    NTG = N_CORES * NT

    key = gtile * 2 + half
    order = np.argsort(key, kind="stable")
    counts = np.bincount(key, minlength=NTG * 2)
    NBL = max(1, int(np.ceil(counts[0::2].max() / P)))
    NBH = max(1, int(np.ceil(counts[1::2].max() / P)))
    NBT = NBL + NBH
    starts = np.zeros(NTG * 2 + 1, np.int64)
    np.cumsum(counts, out=starts[1:])

    skey = key[order]
    pos = np.arange(E, dtype=np.int64) - starts[skey]
    shalf = half[order]
    sg = gtile[order]
    blk = pos // P + shalf * NBL             # block column in [0, NBT)
    prt = pos % P
    row = sg * P + prt

    slot_arr = np.zeros((NTG * P, NBT), np.float32)
    w_arr = np.zeros((NTG * P, NBT), np.float32)
    slot_arr[row, blk] = slot[order]
    w_arr[row, blk] = ev[order]

    # linear per-(tile,half) gather index lists, padded with 0
    lin_lo = np.zeros((NTG, NBL * P), np.int16)
    lin_hi = np.zeros((NTG, NBH * P), np.int16)
    sidx = (src[order] - shalf * SPLIT).astype(np.int16)
    lo_m = shalf == 0
    lin_lo[sg[lo_m], pos[lo_m]] = sidx[lo_m]
    hi_m = ~lo_m
    lin_hi[sg[hi_m], pos[hi_m]] = sidx[hi_m]
    idx16_lo = _wrap_idx16(lin_lo, NTG, NBL)
    idx16_hi = _wrap_idx16(lin_hi, NTG, NBH)

    rows_pc = NT * P
    in_maps = [
        {
            "xt": XT,
            "w": Wb,
            "eidxlo": idx16_lo[c * rows_pc : (c + 1) * rows_pc],
            "eidxhi": idx16_hi[c * rows_pc : (c + 1) * rows_pc],
            "eslot": slot_arr[c * rows_pc : (c + 1) * rows_pc],
            "ew": w_arr[c * rows_pc : (c + 1) * rows_pc],
        }
        for c in range(N_CORES)
    ]
    nc = _get_nc(NBL, NBH)
    return nc, in_maps


def kernel(X, W, edge_src, edge_dst, edge_vals):
    from concourse.bass_utils import run_bass_kernel_spmd

    nc, in_maps = prepare(X, W, edge_src, edge_dst, edge_vals)
    res = run_bass_kernel_spmd(nc, in_maps, core_ids=list(range(N_CORES)))
    outs = res.results
    full = np.empty((N_NODES, H_OUT), np.float32)
    for c in range(N_CORES):
        full[c * DPC : (c + 1) * DPC] = np.asarray(outs[c]["out"])[:DPC]
    return full
